# revision 16
# baseline (speedup 1.0000x reference)
"""Additive attention kernel for Trainium2, data-parallel over batch on 8 cores.

Computation (per batch b):
  x3 = W_conv1 @ x_b            # [HID, HW]  (1x1 conv, contract over C)
  h3 = W_lin @ h_b + b_lin      # [HID]
  a1 = tanh(x3 + h3[:, None])   # [HID, HW]
  a2 = W_attn @ a1              # [HW]
  a3 = softmax(a2)              # [HW]
  ctx = x_b @ a3                # [C]
Outputs: (a3 [B, HW], ctx [B, C]).

Per-core mapping (4 batches/core):
  - conv on PE: lhsT = W_conv1.T chunks [128(C), 128(HID)], rhs = x chunks
    [128(C), 512(HW)], PSUM-accumulated over 4 C-chunks. float32r (or bf16)
    operands run ~2.5x (resp. 4x) the fp32 matmul rate.
  - tanh+bias on ACT (bias = h3 per-partition column).
  - a2 on PE: lhsT = W_attn chunk [128, 1], rhs = a1 [128, 512], M=1 row.
  - softmax per batch WITHOUT max subtraction (|a2| <~ 16, exp safe in f32):
    ACT exp (accum_out -> Z), DVE reciprocal, ACT scale-copy.
  - a3 broadcast across partitions via PE ones-matmul, copied PSUM->SBUF on
    DVE so GpSimd (which cannot touch PSUM) can read it.
  - ctx: per C-chunk multiply on GpSimd, free-dim reduce on DVE.
  - a PE spinner warms the HAM clock gate during the initial x DMA.
"""

import os
import sys

sys.path.insert(0, "/opt/trn_rl_repo")

from contextlib import ExitStack

import ml_dtypes
import numpy as np

import concourse.bacc as bacc
import concourse.mybir as mybir
import concourse.tile as tile
from concourse.bass_utils import run_bass_kernel_spmd

B, C, HGT, WID = 32, 512, 32, 32
HW = HGT * WID
HID, MEM = 256, 256
NCORES = 8
BL = B // NCORES  # batches per core
P = 128
KC = C // P       # C chunks (contract dim of conv)
MC = HID // P     # HID chunks
CC = C // P       # C chunks of ctx output
NSPLIT = 512      # moving-operand free dim per matmul (one PSUM bank of f32)
NH = HW // NSPLIT
NSPIN = int(os.environ.get("ADDATT_NSPIN", "40"))

F32 = mybir.dt.float32
F32R = mybir.dt.float32r
BF16 = mybir.dt.bfloat16
PREC = os.environ.get("ADDATT_PREC", "f32r")  # f32r | bf16

_CACHE = {}


def _ew(ap):
    """View for elementwise engines: f32r bits are plain f32."""
    return ap.bitcast(F32) if ap.dtype == F32R else ap


def build_nc():
    xdt = BF16 if PREC == "bf16" else F32R    # x / w1t / wa / a1 dtype
    np_xdt = ml_dtypes.bfloat16 if PREC == "bf16" else np.float32

    nc = bacc.Bacc("TRN2", target_bir_lowering=False, debug=False,
                   num_devices=NCORES)
    x_d = nc.dram_tensor("x", [BL, C, HW], xdt, kind="ExternalInput").ap()
    w1t_d = nc.dram_tensor("w1t", [C, HID], xdt, kind="ExternalInput").ap()
    wa_d = nc.dram_tensor("wa", [HID, 1], xdt, kind="ExternalInput").ap()
    wlt_d = nc.dram_tensor("wlt", [MEM, HID], F32, kind="ExternalInput").ap()
    ht_d = nc.dram_tensor("ht", [MEM, BL], F32, kind="ExternalInput").ap()
    bl_d = nc.dram_tensor("bl", [HID, 1], F32, kind="ExternalInput").ap()
    ones_d = nc.dram_tensor("ones", [1, NSPLIT], F32R, kind="ExternalInput").ap()
    a3_d = nc.dram_tensor("a3", [BL, HW], F32, kind="ExternalOutput").ap()
    ctx_d = nc.dram_tensor("ctx", [BL, C], F32, kind="ExternalOutput").ap()

    Act = mybir.ActivationFunctionType

    with tile.TileContext(nc) as tc, ExitStack() as ctx:
        consts = ctx.enter_context(tc.tile_pool(name="consts", bufs=1))
        xpool = ctx.enter_context(tc.tile_pool(name="xp", bufs=BL))
        a1pool = ctx.enter_context(tc.tile_pool(name="a1p", bufs=4))
        scrpool = ctx.enter_context(tc.tile_pool(name="scr", bufs=2))
        bcpool = ctx.enter_context(tc.tile_pool(name="bcp", bufs=2))
        smalls = ctx.enter_context(tc.tile_pool(name="smalls", bufs=1))
        convps = ctx.enter_context(tc.tile_pool(name="convps", bufs=2, space="PSUM"))
        a2psp = ctx.enter_context(tc.tile_pool(name="a2psp", bufs=1, space="PSUM"))
        bcpsp = ctx.enter_context(tc.tile_pool(name="bcpsp", bufs=1, space="PSUM"))

        # ---- x loads first: one big DMA per batch on the sync HWDGE ring,
        # so nothing delays the first batch's arrival ----
        xb = []
        for b in range(BL):
            xt = xpool.tile([P, KC, HW], xdt, tag="xb")
            nc.sync.dma_start(out=xt[:], in_=x_d[b].rearrange("(kc p) s -> p kc s", p=P))
            xb.append(xt)

        # ---- weights / constants on the scalar-engine HWDGE ring (parallel
        # with the x stream) ----
        w1t_s = consts.tile([P, KC, HID], xdt)
        nc.scalar.dma_start(out=w1t_s[:], in_=w1t_d.rearrange("(kc p) m -> p kc m", p=P))
        wlt_s = consts.tile([P, MEM // P, HID], F32)
        nc.scalar.dma_start(out=wlt_s[:], in_=wlt_d.rearrange("(kc p) m -> p kc m", p=P))
        ht_s = consts.tile([P, MEM // P, BL], F32)
        nc.scalar.dma_start(out=ht_s[:], in_=ht_d.rearrange("(kc p) b -> p kc b", p=P))
        bl_s = consts.tile([P, MC, 1], F32)
        nc.scalar.dma_start(out=bl_s[:], in_=bl_d.rearrange("(mc p) o -> p mc o", p=P))
        wa_s = consts.tile([P, MC, 1], xdt)
        nc.scalar.dma_start(out=wa_s[:], in_=wa_d.rearrange("(mc p) o -> p mc o", p=P))
        ones_s = consts.tile([1, NSPLIT], F32R)
        nc.scalar.dma_start(out=ones_s[:], in_=ones_d[:])

        # ---- PE warm-up spinner: keeps the HAM activity monitor busy while
        # the first x batch streams in, so real matmuls run at 2.4 GHz ----
        spin = convps.tile([P, NSPLIT], F32, tag="cps")
        for _ in range(NSPIN):
            nc.tensor.matmul(spin[:, :P], lhsT=ones_s[:, :P], rhs=ones_s[:, :P],
                             start=True, stop=True)

        # ---- h3 = W_lin @ h + b_lin (tiny, fp32) ----
        h3_s = smalls.tile([P, MC, BL], F32)
        for mc in range(MC):
            h3ps = convps.tile([P, BL], F32, tag="cps")
            for kc in range(MEM // P):
                nc.tensor.matmul(
                    h3ps[:],
                    lhsT=wlt_s[:, kc, mc * P:(mc + 1) * P],
                    rhs=ht_s[:, kc, :],
                    start=(kc == 0), stop=(kc == MEM // P - 1),
                )
            nc.vector.tensor_scalar_add(h3_s[:, mc, :], h3ps[:], bl_s[:, mc, :])

        # persistent attention-row tiles; every batch uses partition row 0
        # (f32r matmuls at nonzero PSUM base partitions fail codegen) -- the
        # per-batch WAR on row 0 is serialized by Tile and off the hot path
        a2ps = a2psp.tile([P, HW], F32)
        p_sb = smalls.tile([P, HW], F32)
        a3_sb = smalls.tile([P, HW], F32R)
        z_sb = smalls.tile([P, 1], F32)
        rz_sb = smalls.tile([P, 1], F32)

        for b in range(BL):
            # conv + tanh
            a1t = []
            for mc in range(MC):
                cps = convps.tile([P, HW], F32, tag="cps")
                for nh in range(NH):
                    ns = slice(nh * NSPLIT, (nh + 1) * NSPLIT)
                    for kc in range(KC):
                        nc.tensor.matmul(
                            cps[:, ns],
                            lhsT=w1t_s[:, kc, mc * P:(mc + 1) * P],
                            rhs=xb[b][:, kc, ns],
                            start=(kc == 0), stop=(kc == KC - 1),
                        )
                a1 = a1pool.tile([P, HW], xdt, tag="a1")
                nc.scalar.activation(a1[:], cps[:], Act.Tanh,
                                     bias=h3_s[:, mc, b:b + 1])
                a1t.append(a1)

            rb = slice(0, 1)
            # a2 row for this batch
            for nh in range(NH):
                ns = slice(nh * NSPLIT, (nh + 1) * NSPLIT)
                for mc in range(MC):
                    nc.tensor.matmul(
                        a2ps[rb, ns],
                        lhsT=wa_s[:, mc, :],
                        rhs=a1t[mc][:, ns],
                        start=(mc == 0), stop=(mc == MC - 1),
                    )

            # softmax on row b; |a2| is small so no max subtraction needed
            nc.scalar.activation(p_sb[rb, :], a2ps[rb, :], Act.Exp,
                                 accum_out=z_sb[rb, :])
            nc.vector.reciprocal(rz_sb[rb, :], z_sb[rb, :])
            nc.scalar.mul(a3_sb[rb, :], p_sb[rb, :], rz_sb[rb, :])
            nc.sync.dma_start(out=a3_d[b:b + 1, :], in_=_ew(a3_sb[rb, :]))

            # broadcast a3 row across partitions via PE; copy to SBUF so
            # GpSimd (no PSUM access) can read it
            bcps = bcpsp.tile([P, HW], F32, tag="bc")
            for nh in range(NH):
                ns = slice(nh * NSPLIT, (nh + 1) * NSPLIT)
                nc.tensor.matmul(
                    bcps[:, ns],
                    lhsT=ones_s[:, :P],
                    rhs=a3_sb[rb, ns],
                    start=True, stop=True,
                )
            bc_sb = bcpool.tile([P, HW], xdt, tag="bcs")
            nc.vector.tensor_copy(_ew(bc_sb[:]), bcps[:])

            # ctx: multiply on GpSimd, free-dim reduce on DVE
            ctx_sb = smalls.tile([P, CC], F32, tag=f"ctx{b}")
            for cc in range(CC):
                scr = scrpool.tile([P, HW], xdt, tag="scr")
                nc.gpsimd.tensor_mul(_ew(scr[:]), _ew(xb[b][:, cc, :]),
                                     _ew(bc_sb[:]))
                nc.vector.reduce_sum(ctx_sb[:, cc:cc + 1], _ew(scr[:]),
                                     axis=mybir.AxisListType.X)
            nc.sync.dma_start(out=ctx_d[b].rearrange("(cc p) -> p cc", p=P),
                              in_=ctx_sb[:])

    nc.compile()
    return nc, np_xdt


def _get_nc():
    if "nc" not in _CACHE:
        _CACHE["nc"] = build_nc()
    return _CACHE["nc"]


def make_in_maps(x, h, W_conv1, W_lin, b_lin, W_attn, np_xdt):
    x_r = np.asarray(x, np.float32).reshape(B, C, HW).astype(np_xdt)
    w1t = np.ascontiguousarray(np.asarray(W_conv1, np.float32).T).astype(np_xdt)
    wa = np.ascontiguousarray(
        np.asarray(W_attn, np.float32).reshape(HID, 1)).astype(np_xdt)
    wlt = np.ascontiguousarray(np.asarray(W_lin, np.float32).T)
    ht = np.ascontiguousarray(np.asarray(h, np.float32).T)
    bl = np.ascontiguousarray(np.asarray(b_lin, np.float32).reshape(HID, 1))
    ones = np.ones((1, NSPLIT), np.float32)

    in_maps = []
    for i in range(NCORES):
        sl = slice(i * BL, (i + 1) * BL)
        in_maps.append({
            "x": np.ascontiguousarray(x_r[sl]),
            "w1t": w1t,
            "wa": wa,
            "wlt": wlt,
            "ht": np.ascontiguousarray(ht[:, sl]),
            "bl": bl,
            "ones": ones,
        })
    return in_maps


def kernel(x, h, W_conv1, W_lin, b_lin, W_attn):
    nc, np_xdt = _get_nc()
    in_maps = make_in_maps(x, h, W_conv1, W_lin, b_lin, W_attn, np_xdt)
    res = run_bass_kernel_spmd(nc, in_maps, core_ids=list(range(NCORES)))
    a3 = np.concatenate([r["a3"] for r in res.results], axis=0)
    ctx = np.concatenate([r["ctx"] for r in res.results], axis=0)
    return a3, ctx
